# revision 29
# baseline (speedup 1.0000x reference)
"""CirculantLinear as a dense GEMM on 8 TRN2 NeuronCores.

Math: y[b, o] = sum_n x[b, n] * c[o, (-n) mod IN] + bias[o]
    (element 0 of the circular convolution == dot with first row of the
     circulant matrix, vectorized over outputs/batch -> one dense GEMM).

Strategy:
  - Data-parallel over batch: 8 cores x 1024 rows of x each; c/bias replicated.
  - Host-side layout prep (part of sharding): feed each core
      xT  = x_shard.T               [IN, BS]   (contraction-major)
      cT  = c[:, sigma].T           [IN, OUT]  (contraction-major, circulant
                                                column-permutation folded in)
    both cast to fp16 (10-bit mantissa; measured rel err ~1e-4 vs the fp32
    reference at this K=4096 / |c|~1e-2 scale) so DMA bytes halve while the
    tensor engine still runs at full rate (1 row/cycle, same as fp32r).
  - Per core: cache all of xT in SBUF (8.4 MB), stream cT once, accumulate
    out[b:128, o:512] tiles in all 8 PSUM banks, evict via DVE with the
    (partition-broadcast) bias add fused. Each chunk's last k-group runs
    m-major so per-bank accumulation finishes staggered and evictions/stores
    pipeline behind the remaining matmuls instead of piling up at the chunk
    boundary.
  - PE p-state warmup: the TRN2 tensor engine clocks up only after ~3us of
    continuous execution (0.65 -> 1.2 -> 2.4 GHz). A run of matmuls on a
    memset-zero SBUF tile, issued before any DMA-dependent work, rides out
    the ramp while the first weight/x slabs stream in, so every real matmul
    runs at full clock.
  - The first weight DMA is a single k-tile (128 KiB) instead of a 4-tile
    group so real matmuls start ~1us earlier.
"""

import numpy as np

B, OUT, IN = 8192, 4096, 4096
NCORES = 8
BS = B // NCORES  # 1024 batch rows per core
P = 128
KT = IN // P  # 32 contraction tiles
KG = 4  # k-tiles per cT DMA group
N_CHUNK = 512
N_CHUNKS = OUT // N_CHUNK  # 8
M_TILES = BS // P  # 8

_CACHE = {}


def _build_nc(
    reps=1,
    w_bufs=3,
    kg=KG,
    dt16=True,
    n_warm=8,
    warm_ap=256,
    first_kg=2,
    split_rings=True,
    store_ring2=False,
    ragged16=False,
    x_upfront=True,
    mm_groups=1,
):
    """reps>1 repeats the whole compute (idempotent y writes) — used only to
    measure steady-state device time as the slope over reps."""
    import concourse.bacc as bacc
    import concourse.bass as bass
    import concourse.mybir as mybir
    import concourse.tile as tile

    mdt = mybir.dt.float16 if dt16 else mybir.dt.float32r
    nc = bacc.Bacc("TRN2", target_bir_lowering=False, debug=False)
    xT_d = nc.dram_tensor("xT", [IN, BS], mdt, kind="ExternalInput")
    cT_d = nc.dram_tensor("cT", [IN, OUT], mdt, kind="ExternalInput")
    bias_d = nc.dram_tensor("bias", [1, OUT], mybir.dt.float32, kind="ExternalInput")
    y_d = nc.dram_tensor("y", [BS, OUT], mybir.dt.float32, kind="ExternalOutput")

    with tile.TileContext(nc) as tc:
        with (
            tc.tile_pool(name="xpool", bufs=1) as xpool,
            tc.tile_pool(name="wpool", bufs=w_bufs) as wpool,
            tc.tile_pool(name="bpool", bufs=1) as bpool,
            tc.tile_pool(name="opool", bufs=8) as opool,
            tc.tile_pool(name="pspool", bufs=1, space="PSUM") as pspool,
        ):
            # two HWDGE rings: weight stream alone on SP (nc.sync); x preload,
            # bias and output stores on ACT (nc.scalar) so the weight stream —
            # the PE's critical dependency — never queues behind them.
            dma2 = nc.scalar if split_rings else nc.sync
            store_eng = dma2 if store_ring2 else nc.sync

            xT_r = xT_d.ap().rearrange("(ko ki) b -> ki ko b", ki=P)
            cT_r = cT_d.ap().rearrange("(ko ki) o -> ki ko o", ki=P)
            bias_ap = bias_d.ap()

            # PE p-state warmup (see module docstring). ps_warm shares the
            # "ps_0" pool tag, i.e. the same PSUM bank chunk 0 resets at k=0.
            if n_warm:
                wsrc = bpool.tile([P, warm_ap], mdt, name="wsrc")
                nc.vector.memset(wsrc, 0.0)
                ps_warm = pspool.tile([P, N_CHUNK], mybir.dt.float32, name="ps_0")
                for _ in range(n_warm):
                    nc.tensor.matmul(
                        ps_warm[:, :warm_ap],
                        wsrc[:, :P],
                        wsrc,
                        start=True,
                        stop=True,
                    )

            # x cached in SBUF for the whole run; all 32 slab DMAs issued
            # up-front on ring 2 in k order (first-use order), overlapping the
            # weight stream on ring 1.
            xk = [
                xpool.tile([P, BS], mdt, name=f"xk_{ko}") for ko in range(KT)
            ]
            if x_upfront:
                for ko in range(KT):
                    dma2.dma_start(xk[ko], xT_r[:, ko])
            xslice = lambda k, m: xk[k][:, m * P : (m + 1) * P]

            if ragged16:
                # last two chunks 256-wide: halves the eviction/store drain
                # after the final matmul (fp16 matmuls have no narrow-tile
                # rate penalty, unlike fp32r)
                chunks = [(i * N_CHUNK, N_CHUNK) for i in range(N_CHUNKS - 1)]
                chunks += [(OUT - 512, 256), (OUT - 256, 256)]
            else:
                chunks = [(i * N_CHUNK, N_CHUNK) for i in range(N_CHUNKS)]
            for _rep, (n, (o0, ow)) in [
                (r, c) for r in range(reps) for c in enumerate(chunks)
            ]:
                bias_t = bpool.tile([P, N_CHUNK], mybir.dt.float32, name="bias_t")[
                    :, :ow
                ]
                bias_src = bass.AP(
                    tensor=bias_ap.tensor,
                    offset=o0,
                    ap=[[0, P], [1, ow]],
                )
                dma2.dma_start(bias_t, bias_src)

                psums = [
                    pspool.tile([P, N_CHUNK], mybir.dt.float32, name=f"ps_{m}")[
                        :, :ow
                    ]
                    for m in range(M_TILES)
                ]

                # k-tile DMA groups; the very first group of the run is a
                # single tile so the first matmul's dependency is 4x smaller.
                if _rep == 0 and n == 0 and 0 < first_kg < kg:
                    groups = [first_kg, kg - first_kg] + [kg] * (KT // kg - 1)
                else:
                    groups = [kg] * (KT // kg)

                nmm = min(mm_groups, len(groups) - 1)
                head, tail = groups[: len(groups) - nmm], groups[len(groups) - nmm :]

                k0 = 0
                for g in head:
                    w_t = wpool.tile(
                        [P, kg, N_CHUNK], mdt, name="w_t"
                    )[:, :g, :ow]
                    nc.sync.dma_start(
                        w_t,
                        cT_r[:, k0 : k0 + g, o0 : o0 + ow],
                    )
                    if not x_upfront and _rep == 0 and n == 0:
                        # x preload rides along with chunk 0's weight stream;
                        # xk[0] on ring 1 directly behind the first weight
                        # piece (the DMA engine round-robins across rings)
                        for kk in range(g):
                            ko = k0 + kk
                            (nc.sync if ko == 0 else dma2).dma_start(
                                xk[ko], xT_r[:, ko]
                            )
                    for kk in range(g):
                        k = k0 + kk
                        for m in range(M_TILES):
                            nc.tensor.matmul(
                                psums[m],
                                xslice(k, m),
                                w_t[:, kk],
                                start=(k == 0),
                                stop=(k == KT - 1),
                            )
                    k0 += g

                # trailing k-groups m-major: each PSUM bank's accumulation
                # ends staggered (one bank per tail-span of matmuls), so its
                # eviction+store pipelines behind the remaining matmuls
                # instead of piling up at the chunk boundary (PSUM bank reuse
                # would stall the next chunk; at the last chunk this is the
                # final drain).
                w_ts = []
                kts = k0
                for g in tail:
                    w_t = wpool.tile([P, kg, N_CHUNK], mdt, name="w_t")[
                        :, :g, :ow
                    ]
                    nc.sync.dma_start(w_t, cT_r[:, kts : kts + g, o0 : o0 + ow])
                    if not x_upfront and _rep == 0 and n == 0:
                        for kk in range(g):
                            dma2.dma_start(xk[kts + kk], xT_r[:, kts + kk])
                    w_ts.append((w_t, kts, g))
                    kts += g
                for m in range(M_TILES):
                    for w_t, kb, g in w_ts:
                        for kk in range(g):
                            k = kb + kk
                            nc.tensor.matmul(
                                psums[m],
                                xslice(k, m),
                                w_t[:, kk],
                                start=(k == 0),
                                stop=(k == KT - 1),
                            )
                    o_t = opool.tile([P, N_CHUNK], mybir.dt.float32, name="o_t")[
                        :, :ow
                    ]
                    # all evictions on DVE: GPSIMD/Pool cannot read PSUM on
                    # TRN2 (BIR verifier), and in the m-major order DVE
                    # (0.66us per 512-wide add) keeps pace anyway
                    nc.vector.tensor_add(o_t, psums[m], bias_t)
                    store_eng.dma_start(
                        y_d.ap()[m * P : (m + 1) * P, o0 : o0 + ow],
                        o_t,
                    )
    nc.compile()
    return nc


class _Runtime:
    """Compiles the Bass program once and keeps a cached jitted SPMD callable
    (mirrors concourse.bass2jax.run_bass_via_pjrt's multi-core path)."""

    def __init__(self, reps=1, **build_kw):
        import jax
        from jax.experimental.shard_map import shard_map
        from jax.sharding import Mesh, PartitionSpec

        import concourse.mybir as mybir
        from concourse import bass2jax

        bass2jax.install_neuronx_cc_hook()
        nc = _build_nc(reps=reps, **build_kw)
        self.nc = nc

        partition_name = (
            nc.partition_id_tensor.name if nc.partition_id_tensor else None
        )
        in_names = []
        out_names = []
        out_avals = []
        for alloc in nc.m.functions[0].allocations:
            if not isinstance(alloc, mybir.MemoryLocationSet):
                continue
            name = alloc.memorylocations[0].name
            if alloc.kind == "ExternalInput":
                if name != partition_name:
                    in_names.append(name)
            elif alloc.kind == "ExternalOutput":
                out_names.append(name)
                out_avals.append(
                    jax.core.ShapedArray(
                        tuple(alloc.tensor_shape), mybir.dt.np(alloc.dtype)
                    )
                )
        self.in_names = list(in_names)
        self.out_names = out_names
        self.out_avals = out_avals
        n_params = len(in_names)
        n_outs = len(out_names)
        all_names = in_names + out_names
        if partition_name is not None:
            all_names = all_names + [partition_name]

        def _body(*args):
            operands = list(args)
            if partition_name is not None:
                operands.append(bass2jax.partition_id_tensor())
            outs = bass2jax._bass_exec_p.bind(
                *operands,
                out_avals=tuple(out_avals),
                in_names=tuple(all_names),
                out_names=tuple(out_names),
                lowering_input_output_aliases=(),
                sim_require_finite=True,
                sim_require_nnan=True,
                nc=nc,
            )
            return tuple(outs)

        devices = jax.devices()[:NCORES]
        self.mesh = mesh = Mesh(np.asarray(devices), ("core",))
        # xT is batch-sharded along axis 0; cT and bias are replicated
        # (uploaded once, not 8x); outputs are sharded.
        in_specs_by_name = {
            "xT": PartitionSpec("core"),
            "cT": PartitionSpec(),
            "bias": PartitionSpec(),
        }
        in_specs = tuple(in_specs_by_name[n] for n in in_names) + (
            PartitionSpec("core"),
        ) * n_outs
        out_specs = (PartitionSpec("core"),) * n_outs

        def _make_jit():
            return jax.jit(
                shard_map(
                    _body,
                    mesh=mesh,
                    in_specs=in_specs,
                    out_specs=out_specs,
                    check_rep=False,
                ),
                donate_argnums=tuple(range(n_params, n_params + n_outs)),
                keep_unused=True,
            )

        self._make_jit = _make_jit
        self._fn = _make_jit()

    def _zeros(self):
        return [
            np.zeros((NCORES * a.shape[0], *a.shape[1:]), a.dtype)
            for a in self.out_avals
        ]

    def fast_fn(self, example_args):
        """AOT-compiled C++ fast-dispatch variant of _fn (bass_effect
        suppressed) — much lower per-call dispatch overhead."""
        if getattr(self, "_fast", None) is None:
            from concourse import bass2jax

            self._fast = bass2jax.fast_dispatch_compile(
                lambda: self._make_jit().lower(*example_args).compile()
            )
        return self._fast

    def device_inputs(self, xT_all, cT, bias):
        """Pre-place the inputs on the devices with the expected shardings."""
        import jax
        from jax.sharding import NamedSharding, PartitionSpec

        by_name = {"xT": xT_all, "cT": cT, "bias": bias}
        spec_by_name = {
            "xT": PartitionSpec("core"),
            "cT": PartitionSpec(),
            "bias": PartitionSpec(),
        }
        out = [
            jax.device_put(
                by_name[n], NamedSharding(self.mesh, spec_by_name[n])
            )
            for n in self.in_names
        ]
        jax.block_until_ready(out)
        return out

    def run(self, xT_all, cT, bias):
        """xT_all: [NCORES*IN, BS] (core-sharded), cT: [IN, OUT], bias: [1, OUT].
        Returns y [B, OUT]."""
        out_arrs = self._fn(xT_all, cT, bias, *self._zeros())
        (y,) = [np.asarray(a) for a in out_arrs]
        return y

    def timed_call(self, dev_in, fast=True):
        """One timed call with device-resident inputs (zeros staged outside
        the timed region). Returns (seconds, out_arrs)."""
        import time

        import jax
        from jax.sharding import NamedSharding, PartitionSpec

        sh = NamedSharding(self.mesh, PartitionSpec("core"))
        zeros = [jax.device_put(z, sh) for z in self._zeros()]
        jax.block_until_ready(zeros)
        fn = self.fast_fn(tuple(dev_in) + tuple(zeros)) if fast else self._fn
        t0 = time.perf_counter()
        out_arrs = fn(*dev_in, *zeros)
        jax.block_until_ready(out_arrs)
        return time.perf_counter() - t0, out_arrs

    def run_timed(self, dev_in, iters=5, fast=True):
        """Steady-state exec timing with device-resident inputs. Returns
        (times_s, y)."""
        times = []
        out_arrs = None
        for _ in range(iters):
            dt, out_arrs = self.timed_call(dev_in, fast=fast)
            times.append(dt)
        y = np.asarray(out_arrs[0])
        return times, y


def _runtime():
    if "rt" not in _CACHE:
        _CACHE["rt"] = _Runtime()
    return _CACHE["rt"]


def _prep_inputs(x, c, bias):
    """Host-side shard/layout prep: returns (xT_all [8*IN, BS] fp16,
    cT [IN, OUT] fp16, bias [1, OUT] fp32)."""
    x = np.asarray(x, dtype=np.float32)
    c = np.asarray(c, dtype=np.float32)
    bias2 = np.ascontiguousarray(
        np.asarray(bias, dtype=np.float32).reshape(1, OUT)
    )

    sigma = (-np.arange(IN)) % IN
    # cT[nidx, o] = c[o, sigma[nidx]]  (transpose + circulant permutation)
    cT = np.ascontiguousarray(c[:, sigma].T.astype(np.float16))

    # per-core transposed shards, stacked along axis 0 for shard_map
    xT_all = np.ascontiguousarray(
        x.reshape(NCORES, BS, IN)
        .transpose(0, 2, 1)
        .reshape(NCORES * IN, BS)
        .astype(np.float16)
    )
    return xT_all, cT, bias2


def kernel(x, c, bias):
    rt = _runtime()
    xT_all, cT, bias2 = _prep_inputs(x, c, bias)
    try:
        return rt.run(xT_all, cT, bias2)
    except Exception:
        # transient device errors (e.g. a wedged exec unit from an earlier
        # tenant) sometimes clear on retry
        import time as _t

        _t.sleep(2)
        return rt.run(xT_all, cT, bias2)


# revision 31
# speedup vs baseline: 1.0006x; 1.0006x over previous
"""CirculantLinear as a dense GEMM on 8 TRN2 NeuronCores.

Math: y[b, o] = sum_n x[b, n] * c[o, (-n) mod IN] + bias[o]
    (element 0 of the circular convolution == dot with first row of the
     circulant matrix, vectorized over outputs/batch -> one dense GEMM).

Strategy:
  - Data-parallel over batch: 8 cores x 1024 rows of x each; c/bias replicated.
  - Host-side layout prep (part of sharding): feed each core
      xT  = x_shard.T               [IN, BS]   (contraction-major)
      cT  = c[:, sigma].T           [IN, OUT]  (contraction-major, circulant
                                                column-permutation folded in)
    both cast to fp16 (10-bit mantissa; measured rel err ~1e-4 vs the fp32
    reference at this K=4096 / |c|~1e-2 scale) so DMA bytes halve while the
    tensor engine still runs at full rate (1 row/cycle, same as fp32r).
  - Per core: cache all of xT in SBUF (8.4 MB), stream cT once, accumulate
    out[b:128, o:512] tiles in all 8 PSUM banks, evict via DVE with the
    (partition-broadcast) bias add fused. Each chunk's last k-group runs
    m-major so per-bank accumulation finishes staggered and evictions/stores
    pipeline behind the remaining matmuls instead of piling up at the chunk
    boundary.
  - PE p-state warmup: the TRN2 tensor engine clocks up only after ~3us of
    continuous execution (0.65 -> 1.2 -> 2.4 GHz). A run of matmuls on a
    memset-zero SBUF tile, issued before any DMA-dependent work, rides out
    the ramp while the first weight/x slabs stream in, so every real matmul
    runs at full clock.
  - The first weight DMA is a single k-tile (128 KiB) instead of a 4-tile
    group so real matmuls start ~1us earlier.
"""

import numpy as np

B, OUT, IN = 8192, 4096, 4096
NCORES = 8
BS = B // NCORES  # 1024 batch rows per core
P = 128
KT = IN // P  # 32 contraction tiles
KG = 4  # k-tiles per cT DMA group
N_CHUNK = 512
N_CHUNKS = OUT // N_CHUNK  # 8
M_TILES = BS // P  # 8

_CACHE = {}


def _build_nc(
    reps=1,
    w_bufs=3,
    kg=KG,
    dt16=True,
    n_warm=8,
    warm_ap=256,
    kg0=2,
    split_rings=True,
    store_ring2=False,
    ragged16=False,
    x_upfront=True,
    mm_groups=1,
):
    """reps>1 repeats the whole compute (idempotent y writes) — used only to
    measure steady-state device time as the slope over reps."""
    import concourse.bacc as bacc
    import concourse.bass as bass
    import concourse.mybir as mybir
    import concourse.tile as tile

    mdt = mybir.dt.float16 if dt16 else mybir.dt.float32r
    nc = bacc.Bacc("TRN2", target_bir_lowering=False, debug=False)
    xT_d = nc.dram_tensor("xT", [IN, BS], mdt, kind="ExternalInput")
    cT_d = nc.dram_tensor("cT", [IN, OUT], mdt, kind="ExternalInput")
    bias_d = nc.dram_tensor("bias", [1, OUT], mybir.dt.float32, kind="ExternalInput")
    y_d = nc.dram_tensor("y", [BS, OUT], mybir.dt.float32, kind="ExternalOutput")

    with tile.TileContext(nc) as tc:
        with (
            tc.tile_pool(name="xpool", bufs=1) as xpool,
            tc.tile_pool(name="wpool", bufs=w_bufs) as wpool,
            tc.tile_pool(name="bpool", bufs=1) as bpool,
            tc.tile_pool(name="opool", bufs=8) as opool,
            tc.tile_pool(name="pspool", bufs=1, space="PSUM") as pspool,
        ):
            # two HWDGE rings: weight stream alone on SP (nc.sync); x preload,
            # bias and output stores on ACT (nc.scalar) so the weight stream —
            # the PE's critical dependency — never queues behind them.
            dma2 = nc.scalar if split_rings else nc.sync
            store_eng = dma2 if store_ring2 else nc.sync

            xT_r = xT_d.ap().rearrange("(ko ki) b -> ki ko b", ki=P)
            cT_r = cT_d.ap().rearrange("(ko ki) o -> ki ko o", ki=P)
            bias_ap = bias_d.ap()

            # PE p-state warmup (see module docstring). ps_warm shares the
            # "ps_0" pool tag, i.e. the same PSUM bank chunk 0 resets at k=0.
            if n_warm:
                wsrc = bpool.tile([P, warm_ap], mdt, name="wsrc")
                nc.vector.memset(wsrc, 0.0)
                ps_warm = pspool.tile([P, N_CHUNK], mybir.dt.float32, name="ps_0")
                for _ in range(n_warm):
                    nc.tensor.matmul(
                        ps_warm[:, :warm_ap],
                        wsrc[:, :P],
                        wsrc,
                        start=True,
                        stop=True,
                    )

            # x cached in SBUF for the whole run; all 32 slab DMAs issued
            # up-front on ring 2 in k order (first-use order), overlapping the
            # weight stream on ring 1.
            xk = [
                xpool.tile([P, BS], mdt, name=f"xk_{ko}") for ko in range(KT)
            ]
            if x_upfront:
                for ko in range(KT):
                    dma2.dma_start(xk[ko], xT_r[:, ko])
            xslice = lambda k, m: xk[k][:, m * P : (m + 1) * P]

            if ragged16:
                # last two chunks 256-wide: halves the eviction/store drain
                # after the final matmul (fp16 matmuls have no narrow-tile
                # rate penalty, unlike fp32r)
                chunks = [(i * N_CHUNK, N_CHUNK) for i in range(N_CHUNKS - 1)]
                chunks += [(OUT - 512, 256), (OUT - 256, 256)]
            else:
                chunks = [(i * N_CHUNK, N_CHUNK) for i in range(N_CHUNKS)]
            for _rep, (n, (o0, ow)) in [
                (r, c) for r in range(reps) for c in enumerate(chunks)
            ]:
                bias_t = bpool.tile([P, N_CHUNK], mybir.dt.float32, name="bias_t")[
                    :, :ow
                ]
                bias_src = bass.AP(
                    tensor=bias_ap.tensor,
                    offset=o0,
                    ap=[[0, P], [1, ow]],
                )
                dma2.dma_start(bias_t, bias_src)

                psums = [
                    pspool.tile([P, N_CHUNK], mybir.dt.float32, name=f"ps_{m}")[
                        :, :ow
                    ]
                    for m in range(M_TILES)
                ]

                # k-tile DMA groups. Chunk 0 uses smaller groups (kg0): the
                # DMA engine round-robins ring1 (weights) and ring2 (x
                # preload), so chunk 0's delivery cycle is w_group + one x
                # slab; 2-tile w groups keep that cycle (~1.5us) under the
                # PE's 1.7us-per-k consumption, and the first matmul's weight
                # dependency small. Later chunks (x cached) use kg=4.
                if _rep == 0 and n == 0:
                    groups = [kg0] * (KT // kg0)
                else:
                    groups = [kg] * (KT // kg)

                nmm = min(mm_groups, len(groups) - 1)
                head, tail = groups[: len(groups) - nmm], groups[len(groups) - nmm :]

                k0 = 0
                for g in head:
                    w_t = wpool.tile(
                        [P, kg, N_CHUNK], mdt, name="w_t"
                    )[:, :g, :ow]
                    nc.sync.dma_start(
                        w_t,
                        cT_r[:, k0 : k0 + g, o0 : o0 + ow],
                    )
                    if not x_upfront and _rep == 0 and n == 0:
                        # x preload rides along with chunk 0's weight stream;
                        # xk[0] on ring 1 directly behind the first weight
                        # piece (the DMA engine round-robins across rings)
                        for kk in range(g):
                            ko = k0 + kk
                            (nc.sync if ko == 0 else dma2).dma_start(
                                xk[ko], xT_r[:, ko]
                            )
                    for kk in range(g):
                        k = k0 + kk
                        for m in range(M_TILES):
                            nc.tensor.matmul(
                                psums[m],
                                xslice(k, m),
                                w_t[:, kk],
                                start=(k == 0),
                                stop=(k == KT - 1),
                            )
                    k0 += g

                # trailing k-groups m-major: each PSUM bank's accumulation
                # ends staggered (one bank per tail-span of matmuls), so its
                # eviction+store pipelines behind the remaining matmuls
                # instead of piling up at the chunk boundary (PSUM bank reuse
                # would stall the next chunk; at the last chunk this is the
                # final drain).
                w_ts = []
                kts = k0
                for g in tail:
                    w_t = wpool.tile([P, kg, N_CHUNK], mdt, name="w_t")[
                        :, :g, :ow
                    ]
                    nc.sync.dma_start(w_t, cT_r[:, kts : kts + g, o0 : o0 + ow])
                    if not x_upfront and _rep == 0 and n == 0:
                        for kk in range(g):
                            dma2.dma_start(xk[kts + kk], xT_r[:, kts + kk])
                    w_ts.append((w_t, kts, g))
                    kts += g
                for m in range(M_TILES):
                    for w_t, kb, g in w_ts:
                        for kk in range(g):
                            k = kb + kk
                            nc.tensor.matmul(
                                psums[m],
                                xslice(k, m),
                                w_t[:, kk],
                                start=(k == 0),
                                stop=(k == KT - 1),
                            )
                    o_t = opool.tile([P, N_CHUNK], mybir.dt.float32, name="o_t")[
                        :, :ow
                    ]
                    # all evictions on DVE: GPSIMD/Pool cannot read PSUM on
                    # TRN2 (BIR verifier), and in the m-major order DVE
                    # (0.66us per 512-wide add) keeps pace anyway
                    nc.vector.tensor_add(o_t, psums[m], bias_t)
                    store_eng.dma_start(
                        y_d.ap()[m * P : (m + 1) * P, o0 : o0 + ow],
                        o_t,
                    )
    nc.compile()
    return nc


class _Runtime:
    """Compiles the Bass program once and keeps a cached jitted SPMD callable
    (mirrors concourse.bass2jax.run_bass_via_pjrt's multi-core path)."""

    def __init__(self, reps=1, **build_kw):
        import jax
        from jax.experimental.shard_map import shard_map
        from jax.sharding import Mesh, PartitionSpec

        import concourse.mybir as mybir
        from concourse import bass2jax

        bass2jax.install_neuronx_cc_hook()
        nc = _build_nc(reps=reps, **build_kw)
        self.nc = nc

        partition_name = (
            nc.partition_id_tensor.name if nc.partition_id_tensor else None
        )
        in_names = []
        out_names = []
        out_avals = []
        for alloc in nc.m.functions[0].allocations:
            if not isinstance(alloc, mybir.MemoryLocationSet):
                continue
            name = alloc.memorylocations[0].name
            if alloc.kind == "ExternalInput":
                if name != partition_name:
                    in_names.append(name)
            elif alloc.kind == "ExternalOutput":
                out_names.append(name)
                out_avals.append(
                    jax.core.ShapedArray(
                        tuple(alloc.tensor_shape), mybir.dt.np(alloc.dtype)
                    )
                )
        self.in_names = list(in_names)
        self.out_names = out_names
        self.out_avals = out_avals
        n_params = len(in_names)
        n_outs = len(out_names)
        all_names = in_names + out_names
        if partition_name is not None:
            all_names = all_names + [partition_name]

        def _body(*args):
            operands = list(args)
            if partition_name is not None:
                operands.append(bass2jax.partition_id_tensor())
            outs = bass2jax._bass_exec_p.bind(
                *operands,
                out_avals=tuple(out_avals),
                in_names=tuple(all_names),
                out_names=tuple(out_names),
                lowering_input_output_aliases=(),
                sim_require_finite=True,
                sim_require_nnan=True,
                nc=nc,
            )
            return tuple(outs)

        devices = jax.devices()[:NCORES]
        self.mesh = mesh = Mesh(np.asarray(devices), ("core",))
        # xT is batch-sharded along axis 0; cT and bias are replicated
        # (uploaded once, not 8x); outputs are sharded.
        in_specs_by_name = {
            "xT": PartitionSpec("core"),
            "cT": PartitionSpec(),
            "bias": PartitionSpec(),
        }
        in_specs = tuple(in_specs_by_name[n] for n in in_names) + (
            PartitionSpec("core"),
        ) * n_outs
        out_specs = (PartitionSpec("core"),) * n_outs

        def _make_jit():
            return jax.jit(
                shard_map(
                    _body,
                    mesh=mesh,
                    in_specs=in_specs,
                    out_specs=out_specs,
                    check_rep=False,
                ),
                donate_argnums=tuple(range(n_params, n_params + n_outs)),
                keep_unused=True,
            )

        self._make_jit = _make_jit
        self._fn = _make_jit()

    def _zeros(self):
        return [
            np.zeros((NCORES * a.shape[0], *a.shape[1:]), a.dtype)
            for a in self.out_avals
        ]

    def fast_fn(self, example_args):
        """AOT-compiled C++ fast-dispatch variant of _fn (bass_effect
        suppressed) — much lower per-call dispatch overhead."""
        if getattr(self, "_fast", None) is None:
            from concourse import bass2jax

            self._fast = bass2jax.fast_dispatch_compile(
                lambda: self._make_jit().lower(*example_args).compile()
            )
        return self._fast

    def device_inputs(self, xT_all, cT, bias):
        """Pre-place the inputs on the devices with the expected shardings."""
        import jax
        from jax.sharding import NamedSharding, PartitionSpec

        by_name = {"xT": xT_all, "cT": cT, "bias": bias}
        spec_by_name = {
            "xT": PartitionSpec("core"),
            "cT": PartitionSpec(),
            "bias": PartitionSpec(),
        }
        out = [
            jax.device_put(
                by_name[n], NamedSharding(self.mesh, spec_by_name[n])
            )
            for n in self.in_names
        ]
        jax.block_until_ready(out)
        return out

    def run(self, xT_all, cT, bias):
        """xT_all: [NCORES*IN, BS] (core-sharded), cT: [IN, OUT], bias: [1, OUT].
        Returns y [B, OUT]."""
        out_arrs = self._fn(xT_all, cT, bias, *self._zeros())
        (y,) = [np.asarray(a) for a in out_arrs]
        return y

    def timed_call(self, dev_in, fast=True):
        """One timed call with device-resident inputs (zeros staged outside
        the timed region). Returns (seconds, out_arrs)."""
        import time

        import jax
        from jax.sharding import NamedSharding, PartitionSpec

        sh = NamedSharding(self.mesh, PartitionSpec("core"))
        zeros = [jax.device_put(z, sh) for z in self._zeros()]
        jax.block_until_ready(zeros)
        fn = self.fast_fn(tuple(dev_in) + tuple(zeros)) if fast else self._fn
        t0 = time.perf_counter()
        out_arrs = fn(*dev_in, *zeros)
        jax.block_until_ready(out_arrs)
        return time.perf_counter() - t0, out_arrs

    def run_timed(self, dev_in, iters=5, fast=True):
        """Steady-state exec timing with device-resident inputs. Returns
        (times_s, y)."""
        times = []
        out_arrs = None
        for _ in range(iters):
            dt, out_arrs = self.timed_call(dev_in, fast=fast)
            times.append(dt)
        y = np.asarray(out_arrs[0])
        return times, y


def _runtime():
    if "rt" not in _CACHE:
        _CACHE["rt"] = _Runtime()
    return _CACHE["rt"]


def _prep_inputs(x, c, bias):
    """Host-side shard/layout prep: returns (xT_all [8*IN, BS] fp16,
    cT [IN, OUT] fp16, bias [1, OUT] fp32)."""
    x = np.asarray(x, dtype=np.float32)
    c = np.asarray(c, dtype=np.float32)
    bias2 = np.ascontiguousarray(
        np.asarray(bias, dtype=np.float32).reshape(1, OUT)
    )

    sigma = (-np.arange(IN)) % IN
    # cT[nidx, o] = c[o, sigma[nidx]]  (transpose + circulant permutation)
    cT = np.ascontiguousarray(c[:, sigma].T.astype(np.float16))

    # per-core transposed shards, stacked along axis 0 for shard_map
    xT_all = np.ascontiguousarray(
        x.reshape(NCORES, BS, IN)
        .transpose(0, 2, 1)
        .reshape(NCORES * IN, BS)
        .astype(np.float16)
    )
    return xT_all, cT, bias2


def kernel(x, c, bias):
    rt = _runtime()
    xT_all, cT, bias2 = _prep_inputs(x, c, bias)
    try:
        return rt.run(xT_all, cT, bias2)
    except Exception:
        # transient device errors (e.g. a wedged exec unit from an earlier
        # tenant) sometimes clear on retry
        import time as _t

        _t.sleep(2)
        return rt.run(xT_all, cT, bias2)
